# revision 14
# baseline (speedup 1.0000x reference)
"""Trainium2 Bass kernel for nn_DenseTf: out = inputs @ sign(clip(w,-1,1)) + b.

Shapes: inputs [8192, 2048] f32, w [2048, 2048] f32, b [2048] f32 -> [8192, 2048] f32.

Sharding: data-parallel over rows. Each of the 8 NeuronCores gets 1024 rows of
`inputs` staged TRANSPOSED on the host ([2048, 1024] f32, a pure layout choice
for the shard), plus a full replica of `w` and `b`; no collectives. Outputs are
concatenated on the host.

Per-core kernel (bf16 pipeline, ~110us PE floor):
  - All matmul operands are bf16 (x rounded to bf16 ~1e-3 rel err; sign(w) is
    exact in bf16); PSUM accumulation stays f32. PE time matches f32r (1 elem/
    cell/cycle either way) but SBUF residency halves vs f32.
  - The transposed x staging makes xT k-tiles [128, 1024] natural contiguous
    rows: zero on-device transposes (the PE-transpose variant burned ~35us of
    PE; the xbar-DMA variant deadlock-serializes against all other DMAs).
  - One HWDGE ring (sync) carries all HBM reads in k-paired order
    [x0 w0 x1 w1 ... x15 w15] so each k-tile of x and sign(w) arrives together
    (~4.4us cadence); ACT signs w to bf16, DVE casts x to bf16.
  - Schedule fights the 8-PSUM-bank limit during the ~72us read stream with a
    staggered wavefront: 14 waves of 8-group k-segments whose lengths track the
    arrival cadence. Early waves take short k-segments (spilled to bf16 SBUF
    accumulators, bias folded into the first spill), later waves longer ones;
    k-major emission inside a wave lets the leading k run before the trailing
    k has arrived. PE is busy wall-to-wall after ~7us.
"""

import numpy as np

import concourse.bass as bass
import concourse.mybir as mybir
import concourse.tile as tile
from concourse.bass_utils import run_bass_kernel_spmd

N_CORES = 8
N_ROWS, D_IN, D_OUT = 8192, 2048, 2048
ROWS = N_ROWS // N_CORES  # rows per core
P = 128
K_TILES = D_IN // P  # 16
M_TILES = ROWS // P  # 8
NF = 512  # psum bank width in f32
N_TILES = D_OUT // NF  # 4
HALF = D_OUT // 2

F32 = mybir.dt.float32
BF16 = mybir.dt.bfloat16
ADD = mybir.AluOpType.add

# wavefront: (group-set index, k_begin, k_end). Group set i = rows 2i, 2i+1
# (x4 n-tiles = 8 groups = 8 psum banks per wave). Segment lengths are sized
# so each wave's trailing k-tile lands just before the PE cursor reaches it.
# By wave 9 the whole wire has arrived: those waves run chain-major with
# per-group retire so the DVE adds overlap the next group's matmuls (kills
# the 8-serial-ADD tail after the last matmul).
WAVES = [
    (0, 0, 1), (1, 0, 2), (2, 0, 3), (3, 0, 4),
    (0, 1, 5), (1, 2, 6), (2, 3, 8), (3, 4, 9),
    (0, 5, 10), (1, 6, 11), (2, 8, 16), (3, 9, 16),
    (0, 10, 16), (1, 11, 16),
]
N_CHAIN_MAJOR = 5  # waves [5:] run chain-major


def _split_waits_pass(nc, max_waits=1):
    """Cap semaphore waits per instruction for this container's walrus.

    The pinned walrus errors ("Too many sync wait commands") when an
    instruction carries more than ~2 sync waits. Move overflow waits onto
    same-engine NoOps inserted immediately before the instruction; the engine
    executes its stream in order, so the gating semantics are identical.
    """
    idx = 0
    for f in nc.m.functions:
        for bb in f.blocks:
            insts = list(bb.instructions)
            changed = False
            out = []
            for inst in insts:
                si = inst.sync_info
                if si is not None and si.on_wait and len(si.on_wait) > max_waits:
                    waits = list(si.on_wait)
                    keep, rest = waits[:max_waits], waits[max_waits:]
                    for i in range(0, len(rest), max_waits):
                        nop = mybir.InstNoOp(
                            name=f"splitw-{idx}",
                            ins=[],
                            outs=[],
                            engine=inst.engine,
                            sync_info=mybir.SyncInfo(
                                on_wait=rest[i : i + max_waits], on_update=[]
                            ),
                        )
                        idx += 1
                        out.append(nop)
                    inst.sync_info = mybir.SyncInfo(
                        on_wait=keep, on_update=list(si.on_update or [])
                    )
                    changed = True
                out.append(inst)
            if changed:
                bb.instructions.clear()
                bb.instructions.extend(out)


def _build_nc():
    nc = bass.Bass()
    xt_d = nc.dram_tensor("xs", [D_IN, ROWS], F32, kind="ExternalInput")
    w_d = nc.dram_tensor("w", [D_IN, D_OUT], F32, kind="ExternalInput")
    b_d = nc.dram_tensor("b", [D_OUT], F32, kind="ExternalInput")
    y_d = nc.dram_tensor("y", [ROWS, D_OUT], F32, kind="ExternalOutput")

    with tile.TileContext(nc) as tc:
        with (
            tc.tile_pool(name="const", bufs=1) as const,
            tc.tile_pool(name="s", bufs=K_TILES) as s_pool,
            tc.tile_pool(name="xb", bufs=K_TILES) as xb_pool,
            tc.tile_pool(name="wstage", bufs=3) as wstage,
            tc.tile_pool(name="xstage", bufs=2) as xstage,
            tc.tile_pool(name="acc", bufs=32) as acc_pool,
            tc.tile_pool(name="y", bufs=6) as y_pool,
            tc.tile_pool(name="psy", bufs=8, space="PSUM") as psum_y,
        ):
            b_bcast = const.tile([P, D_OUT], F32)

            # PE warmup: ~32 dummy matmuls on zeroed tiles fill the PE stream
            # ahead of the first gated matmul, flipping the HAM clock gate to
            # 8/8 and paying the cold-start ramp on throwaway work
            warm_l = const.tile([P, P], BF16)
            warm_r = const.tile([P, NF], BF16)
            nc.vector.memset(warm_l[:], 0)
            nc.vector.memset(warm_r[:], 0)
            warm_ps = psum_y.tile([P, NF], F32, name="warm_ps", tag="psy")
            for _ in range(40):
                nc.tensor.matmul(warm_ps[:], warm_l[:], warm_r[:], start=True, stop=True)

            s = {}
            xb = {}

            # k-paired wire: xT k-tile then w k-tile, all on the sync ring.
            # First w tile is split into half-DMAs so its sign (and the first
            # real matmuls) start ~1.5us earlier.
            for k in range(K_TILES):
                wt = wstage.tile([P, D_OUT], F32, name=f"wt{k}", tag="wt")
                st = s_pool.tile([P, D_OUT], BF16, name=f"s{k}", tag="s")
                if k == 0:
                    # w half first on the wire: the w->sign path is longer
                    # than the x->cast path, and sign h0 (slice-level
                    # dependency) starts after only 512KB
                    nc.sync.dma_start(wt[:, :HALF], w_d[k * P : (k + 1) * P, :HALF])

                xa = xstage.tile([P, ROWS], F32, name=f"xa{k}", tag="xa")
                nc.sync.dma_start(xa[:], xt_d[k * P : (k + 1) * P, :])
                xb[k] = xb_pool.tile([P, ROWS], BF16, name=f"xb{k}", tag="xb")
                nc.vector.tensor_copy(xb[k][:], xa[:])

                if k == 0:
                    nc.sync.dma_start(wt[:, HALF:], w_d[k * P : (k + 1) * P, HALF:])
                else:
                    nc.sync.dma_start(wt[:], w_d[k * P : (k + 1) * P, :])
                for h in range(2):
                    hs = slice(h * HALF, (h + 1) * HALF)
                    nc.scalar.activation(
                        st[:, hs], wt[:, hs], mybir.ActivationFunctionType.Sign
                    )
                s[k] = st
                if k == 2:
                    # bias broadcast (gpsimd ring); emitted here so it doesn't
                    # steal wire bandwidth from the first x/w tiles
                    nc.gpsimd.dma_start(
                        b_bcast[:], b_d[None, :].to_broadcast([P, D_OUT])
                    )

            acc = {}

            def retire(g, ps_g, k0, k1):
                m, n = g
                if k0 == 0:
                    # first segment: spill to bf16 accumulator, bias folded
                    acc[g] = acc_pool.tile([P, NF], BF16, name=f"acc{m}_{n}", tag="acc")
                    nc.vector.tensor_tensor(
                        acc[g][:], ps_g[:], b_bcast[:, n * NF : (n + 1) * NF], ADD
                    )
                elif k1 < K_TILES:
                    nc.vector.tensor_tensor(acc[g][:], ps_g[:], acc[g][:], ADD)
                else:
                    yt = y_pool.tile([P, NF], F32, name=f"y{m}_{n}", tag="y")
                    nc.vector.tensor_tensor(yt[:], ps_g[:], acc[g][:], ADD)
                    # sync ring is free once the reads are dispatched; HWDGE
                    # gen is ~2x faster than SWDGE for the tail stores
                    nc.sync.dma_start(
                        y_d[m * P : (m + 1) * P, n * NF : (n + 1) * NF], yt[:]
                    )

            for wi, (gi, k0, k1) in enumerate(WAVES):
                groups = [(m, n) for m in (2 * gi, 2 * gi + 1) for n in range(N_TILES)]
                if wi == 0:
                    # first wave: n-major so the matmuls gated only on the
                    # first w half-tile (n 0..1) issue first
                    groups = sorted(groups, key=lambda g: g[1])
                if wi < N_CHAIN_MAJOR:
                    # k-major: the wave's leading k-tiles run as soon as they
                    # have arrived; only the trailing k gates the wave's tail
                    ps = {}
                    for m, n in groups:
                        ps[(m, n)] = psum_y.tile(
                            [P, NF], F32, name=f"ps{m}_{n}_{k0}", tag="psy"
                        )
                    for k in range(k0, k1):
                        for m, n in groups:
                            nc.tensor.matmul(
                                ps[(m, n)][:],
                                xb[k][:, m * P : (m + 1) * P],
                                s[k][:, n * NF : (n + 1) * NF],
                                start=(k == k0),
                                stop=(k == k1 - 1),
                            )
                    for g in groups:
                        retire(g, ps[g], k0, k1)
                else:
                    # chain-major with per-group retire: everything is resident
                    # by now, and each group's ADD overlaps the next group's MMs
                    for m, n in groups:
                        ps_g = psum_y.tile(
                            [P, NF], F32, name=f"ps{m}_{n}_{k0}", tag="psy"
                        )
                        for k in range(k0, k1):
                            nc.tensor.matmul(
                                ps_g[:],
                                xb[k][:, m * P : (m + 1) * P],
                                s[k][:, n * NF : (n + 1) * NF],
                                start=(k == k0),
                                stop=(k == k1 - 1),
                            )
                        retire((m, n), ps_g, k0, k1)

    _split_waits_pass(nc, max_waits=1)
    return nc


_NC_CACHE = None


def _get_nc():
    global _NC_CACHE
    if _NC_CACHE is None:
        _NC_CACHE = _build_nc()
    return _NC_CACHE


def _run(inputs, w, b, trace=False):
    nc = _get_nc()
    inputs = np.asarray(inputs, dtype=np.float32)
    w = np.ascontiguousarray(w, dtype=np.float32)
    b = np.ascontiguousarray(b, dtype=np.float32)
    in_maps = [
        {
            # shard rows, stage transposed (layout choice for the k-tiled load)
            "xs": np.ascontiguousarray(inputs[i * ROWS : (i + 1) * ROWS].T),
            "w": w,
            "b": b,
        }
        for i in range(N_CORES)
    ]
    res = run_bass_kernel_spmd(nc, in_maps, list(range(N_CORES)), trace=trace)
    out = np.concatenate([res.results[i]["y"] for i in range(N_CORES)], axis=0)
    return out, res


def kernel(inputs, w, b):
    out, _ = _run(inputs, w, b, trace=False)
    return out


# revision 16
# speedup vs baseline: 1.0375x; 1.0375x over previous
"""Trainium2 Bass kernel for nn_DenseTf: out = inputs @ sign(clip(w,-1,1)) + b.

Shapes: inputs [8192, 2048] f32, w [2048, 2048] f32, b [2048] f32 -> [8192, 2048] f32.

Sharding: data-parallel over rows. Each of the 8 NeuronCores gets 1024 rows of
`inputs` staged TRANSPOSED on the host ([2048, 1024] f32, a pure layout choice
for the shard), plus a full replica of `w` and `b`; no collectives. Outputs are
concatenated on the host.

Per-core kernel (bf16 pipeline, ~110us PE floor):
  - All matmul operands are bf16 (x rounded to bf16 ~1e-3 rel err; sign(w) is
    exact in bf16); PSUM accumulation stays f32. PE time matches f32r (1 elem/
    cell/cycle either way) but SBUF residency halves vs f32.
  - The transposed x staging makes xT k-tiles [128, 1024] natural contiguous
    rows: zero on-device transposes (the PE-transpose variant burned ~35us of
    PE; the xbar-DMA variant deadlock-serializes against all other DMAs).
  - One HWDGE ring (sync) carries all HBM reads in k-paired order
    [x0 w0 x1 w1 ... x15 w15] so each k-tile of x and sign(w) arrives together
    (~4.4us cadence); ACT signs w to bf16, DVE casts x to bf16.
  - Schedule fights the 8-PSUM-bank limit during the ~72us read stream with a
    staggered wavefront: 14 waves of 8-group k-segments whose lengths track the
    arrival cadence. Early waves take short k-segments (spilled to bf16 SBUF
    accumulators, bias folded into the first spill), later waves longer ones;
    k-major emission inside a wave lets the leading k run before the trailing
    k has arrived. PE is busy wall-to-wall after ~7us.
"""

import numpy as np

import concourse.bass as bass
import concourse.mybir as mybir
import concourse.tile as tile
from concourse.bass_utils import run_bass_kernel_spmd

N_CORES = 8
N_ROWS, D_IN, D_OUT = 8192, 2048, 2048
ROWS = N_ROWS // N_CORES  # rows per core
P = 128
K_TILES = D_IN // P  # 16
M_TILES = ROWS // P  # 8
NF = 512  # psum bank width in f32
N_TILES = D_OUT // NF  # 4
HALF = D_OUT // 2

F32 = mybir.dt.float32
BF16 = mybir.dt.bfloat16
ADD = mybir.AluOpType.add

# wavefront: (group-set index, k_begin, k_end). Group set i = rows 2i, 2i+1
# (x4 n-tiles = 8 groups = 8 psum banks per wave). Segment lengths are sized
# so each wave's trailing k-tile lands just before the PE cursor reaches it.
# By wave 9 the whole wire has arrived: those waves run chain-major with
# per-group retire so the DVE adds overlap the next group's matmuls (kills
# the 8-serial-ADD tail after the last matmul).
WAVES = [
    (0, 0, 1), (1, 0, 2), (2, 0, 3), (3, 0, 4),
    (0, 1, 5), (1, 2, 6), (2, 3, 8), (3, 4, 9),
    (0, 5, 10), (1, 6, 11), (2, 8, 16), (3, 9, 16),
    (0, 10, 16), (1, 11, 16),
]
N_CHAIN_MAJOR = 5  # waves [5:] run chain-major


def _split_waits_pass(nc, max_waits=1):
    """Cap semaphore waits per instruction for this container's walrus.

    The pinned walrus errors ("Too many sync wait commands") when an
    instruction carries more than ~2 sync waits. Move overflow waits onto
    same-engine NoOps inserted immediately before the instruction; the engine
    executes its stream in order, so the gating semantics are identical.
    """
    idx = 0
    for f in nc.m.functions:
        for bb in f.blocks:
            insts = list(bb.instructions)
            changed = False
            out = []
            for inst in insts:
                si = inst.sync_info
                if si is not None and si.on_wait and len(si.on_wait) > max_waits:
                    waits = list(si.on_wait)
                    keep, rest = waits[:max_waits], waits[max_waits:]
                    for i in range(0, len(rest), max_waits):
                        nop = mybir.InstNoOp(
                            name=f"splitw-{idx}",
                            ins=[],
                            outs=[],
                            engine=inst.engine,
                            sync_info=mybir.SyncInfo(
                                on_wait=rest[i : i + max_waits], on_update=[]
                            ),
                        )
                        idx += 1
                        out.append(nop)
                    inst.sync_info = mybir.SyncInfo(
                        on_wait=keep, on_update=list(si.on_update or [])
                    )
                    changed = True
                out.append(inst)
            if changed:
                bb.instructions.clear()
                bb.instructions.extend(out)


def _build_nc():
    nc = bass.Bass()
    xt_d = nc.dram_tensor("xs", [D_IN, ROWS], F32, kind="ExternalInput")
    w_d = nc.dram_tensor("w", [D_IN, D_OUT], F32, kind="ExternalInput")
    b_d = nc.dram_tensor("b", [D_OUT], F32, kind="ExternalInput")
    y_d = nc.dram_tensor("y", [ROWS, D_OUT], F32, kind="ExternalOutput")

    with tile.TileContext(nc) as tc:
        with (
            tc.tile_pool(name="const", bufs=1) as const,
            tc.tile_pool(name="s", bufs=K_TILES) as s_pool,
            tc.tile_pool(name="xb", bufs=K_TILES) as xb_pool,
            tc.tile_pool(name="wstage", bufs=3) as wstage,
            tc.tile_pool(name="xstage", bufs=2) as xstage,
            tc.tile_pool(name="acc", bufs=32) as acc_pool,
            tc.tile_pool(name="y", bufs=6) as y_pool,
            tc.tile_pool(name="psy", bufs=8, space="PSUM") as psum_y,
        ):
            b_bcast = const.tile([P, D_OUT], F32)

            # PE warmup: ~32 dummy matmuls on zeroed tiles fill the PE stream
            # ahead of the first gated matmul, flipping the HAM clock gate to
            # 8/8 and paying the cold-start ramp on throwaway work
            warm_l = const.tile([P, P], BF16)
            warm_r = const.tile([P, NF], BF16)
            nc.vector.memset(warm_l[:], 0)
            nc.vector.memset(warm_r[:], 0)
            warm_ps = psum_y.tile([P, NF], F32, name="warm_ps", tag="psy")
            for _ in range(40):
                nc.tensor.matmul(warm_ps[:], warm_l[:], warm_r[:], start=True, stop=True)

            s = {}
            xb = {}

            # k-paired wire: xT k-tile then w k-tile, all on the sync ring.
            # First w tile is split into half-DMAs so its sign (and the first
            # real matmuls) start ~1.5us earlier.
            for k in range(K_TILES):
                xa = xstage.tile([P, ROWS], F32, name=f"xa{k}", tag="xa")
                nc.sync.dma_start(xa[:], xt_d[k * P : (k + 1) * P, :])
                xb[k] = xb_pool.tile([P, ROWS], BF16, name=f"xb{k}", tag="xb")
                nc.vector.tensor_copy(xb[k][:], xa[:])

                wt = wstage.tile([P, D_OUT], F32, name=f"wt{k}", tag="wt")
                st = s_pool.tile([P, D_OUT], BF16, name=f"s{k}", tag="s")
                if k == 0:
                    # split first w tile into half-DMAs: sign h0 (slice-level
                    # dependency) starts after only 512KB, not the full 1MB
                    for h in range(2):
                        hs = slice(h * HALF, (h + 1) * HALF)
                        nc.sync.dma_start(wt[:, hs], w_d[k * P : (k + 1) * P, hs])
                else:
                    nc.sync.dma_start(wt[:], w_d[k * P : (k + 1) * P, :])
                for h in range(2):
                    hs = slice(h * HALF, (h + 1) * HALF)
                    nc.scalar.activation(
                        st[:, hs], wt[:, hs], mybir.ActivationFunctionType.Sign
                    )
                s[k] = st
                if k == 2:
                    # bias broadcast (gpsimd ring); emitted here so it doesn't
                    # steal wire bandwidth from the first x/w tiles
                    nc.gpsimd.dma_start(
                        b_bcast[:], b_d[None, :].to_broadcast([P, D_OUT])
                    )

            acc = {}

            def retire(g, ps_g, k0, k1):
                m, n = g
                if k0 == 0:
                    # first segment: spill to bf16 accumulator, bias folded
                    acc[g] = acc_pool.tile([P, NF], BF16, name=f"acc{m}_{n}", tag="acc")
                    nc.vector.tensor_tensor(
                        acc[g][:], ps_g[:], b_bcast[:, n * NF : (n + 1) * NF], ADD
                    )
                elif k1 < K_TILES:
                    nc.vector.tensor_tensor(acc[g][:], ps_g[:], acc[g][:], ADD)
                else:
                    yt = y_pool.tile([P, NF], F32, name=f"y{m}_{n}", tag="y")
                    nc.vector.tensor_tensor(yt[:], ps_g[:], acc[g][:], ADD)
                    # sync ring is free once the reads are dispatched; HWDGE
                    # gen is ~2x faster than SWDGE for the tail stores
                    nc.sync.dma_start(
                        y_d[m * P : (m + 1) * P, n * NF : (n + 1) * NF], yt[:]
                    )

            for wi, (gi, k0, k1) in enumerate(WAVES):
                groups = [(m, n) for m in (2 * gi, 2 * gi + 1) for n in range(N_TILES)]
                if wi < N_CHAIN_MAJOR:
                    # k-major: the wave's leading k-tiles run as soon as they
                    # have arrived; only the trailing k gates the wave's tail
                    ps = {}
                    for m, n in groups:
                        ps[(m, n)] = psum_y.tile(
                            [P, NF], F32, name=f"ps{m}_{n}_{k0}", tag="psy"
                        )
                    for k in range(k0, k1):
                        for m, n in groups:
                            nc.tensor.matmul(
                                ps[(m, n)][:],
                                xb[k][:, m * P : (m + 1) * P],
                                s[k][:, n * NF : (n + 1) * NF],
                                start=(k == k0),
                                stop=(k == k1 - 1),
                            )
                    for g in groups:
                        retire(g, ps[g], k0, k1)
                else:
                    # chain-major with per-group retire: everything is resident
                    # by now, and each group's ADD overlaps the next group's MMs
                    for m, n in groups:
                        ps_g = psum_y.tile(
                            [P, NF], F32, name=f"ps{m}_{n}_{k0}", tag="psy"
                        )
                        for k in range(k0, k1):
                            nc.tensor.matmul(
                                ps_g[:],
                                xb[k][:, m * P : (m + 1) * P],
                                s[k][:, n * NF : (n + 1) * NF],
                                start=(k == k0),
                                stop=(k == k1 - 1),
                            )
                        retire((m, n), ps_g, k0, k1)

    _split_waits_pass(nc, max_waits=1)
    return nc


_NC_CACHE = None


def _get_nc():
    global _NC_CACHE
    if _NC_CACHE is None:
        _NC_CACHE = _build_nc()
    return _NC_CACHE


def _run(inputs, w, b, trace=False):
    nc = _get_nc()
    inputs = np.asarray(inputs, dtype=np.float32)
    w = np.ascontiguousarray(w, dtype=np.float32)
    b = np.ascontiguousarray(b, dtype=np.float32)
    in_maps = [
        {
            # shard rows, stage transposed (layout choice for the k-tiled load)
            "xs": np.ascontiguousarray(inputs[i * ROWS : (i + 1) * ROWS].T),
            "w": w,
            "b": b,
        }
        for i in range(N_CORES)
    ]
    res = run_bass_kernel_spmd(nc, in_maps, list(range(N_CORES)), trace=trace)
    out = np.concatenate([res.results[i]["y"] for i in range(N_CORES)], axis=0)
    return out, res


def kernel(inputs, w, b):
    out, _ = _run(inputs, w, b, trace=False)
    return out
